# revision 24
# baseline (speedup 1.0000x reference)
"""Trainium2 Bass kernel for nn_Aggregation_74904229642960 (gnn_message_passing).

The reference computes, with tgt = edge_index[1]:

    sm  = segment_softmax(x, tgt, N)   # per-(target node, feature) softmax over edges
    out = segment_sum(sm, tgt, N)      # [N, d]

The final segment_sum contracts exactly the segments the softmax normalized
over, and softmax weights sum to 1 over their own segment.  Hence, exactly
(independent of x, which only shifts/scales terms that cancel):

    out[n, f] = 1.0  if node n has >= 1 incoming edge, else 0.0

(The fp32 reference deviates from 1.0 by < 1e-6 rounding noise.)  The optimal
kernel therefore reads only edge_index[1]: it computes the in-degree histogram
(bincount over the 10000 nodes) on device and emits 1.0 rows for nodes with
nonzero degree.

Sharding (8 NeuronCores): edges are split E/8 per core (the E dim of
edge_index), each core builds a partial per-node histogram, the partials are
combined with a ReduceScatter(add) collective, and each core writes its 1/8
slice of the [N, d] output, which the host concatenates.

Per-core bincount (E_loc = 80000 edges), using node id n = hi*128 + lo:
  for each tile of 128 edges (one edge per SBUF partition):
      A[e, :] = onehot80(hi_e)    # DVE is_equal against iota, bf16
      B[e, :] = onehot128(lo_e)
      counts[hi, lo] += A^T @ B   # PE matmul, fp32 PSUM accumulation
  counts[hi, lo] == #edges with target hi*128+lo   (exact: 0/1 products,
  fp32 accumulate, counts <= 80000 << 2^24)
"""

import os

import numpy as np

import concourse.bass as bass
import concourse.mybir as mybir
import concourse.tile as tile
from concourse.bass_utils import run_bass_kernel_spmd
from concourse.masks import make_identity

N_NODES = 10000
N_EDGES = 640000
D_FEAT = 128
N_CORES = 8

P = 128               # SBUF partitions / edges per tile
HI = 80               # hi-digit one-hot width (hi = n >> 7 in [0, 79))
LO = 128              # lo-digit one-hot width (lo = n & 127)
NODES_PAD = HI * LO   # 10240 >= N_NODES
ROWS_PER_CORE = NODES_PAD // N_CORES      # 1280 output rows per core
OUT_TILES = ROWS_PER_CORE // P            # 10 output tiles of 128 nodes

E_LOC = N_EDGES // N_CORES                # 80000 edges per core
NT = E_LOC // P                           # 625 edge tiles per core
E_PAD = NT * P
PAD_NODE = 79 * LO                        # padding target (>= N_NODES, host-trimmed)
POOL_EVERY = 3                            # every 3rd tile's one-hots on GpSimd

f32 = mybir.dt.float32
bf16 = mybir.dt.bfloat16
i16 = mybir.dt.int16
i32 = mybir.dt.int32

# run_bass_kernel_spmd results of the most recent kernel() call (for test
# harness introspection: exec_time_ns etc. when BASS_TRACE=1).
LAST_RESULTS = None


def _ensure_ntff_hook():
    """Install the axon NTFF-profile hook if the container's antenv stub
    lacks it (profiling-only; kernel correctness does not depend on this)."""
    import sys
    import types

    try:
        from antenv.axon_hooks import get_axon_ntff_profile_hook  # noqa: F401

        return
    except ImportError:
        pass
    m = types.ModuleType("antenv.axon_hooks")
    m._hook = None
    m.set_axon_ntff_profile_hook = lambda h: setattr(m, "_hook", h)
    m.get_axon_ntff_profile_hook = lambda: m._hook
    import antenv

    sys.modules["antenv.axon_hooks"] = m
    antenv.axon_hooks = m
    try:
        from trn_agent_boot.trn_boot import _ntff_profile_via_ctypes

        hook = _ntff_profile_via_ctypes("/opt/axon/libaxon_pjrt.so")
        if hook is not None:
            m._hook = hook
    except Exception as e:  # profiling is best-effort
        print("ntff hook install failed:", e)


_ENGINE_SEM_PREFIX = {
    mybir.EngineType.PE: "PE_",
    mybir.EngineType.DVE: "DVE_",
    mybir.EngineType.Activation: "ACT_",
    mybir.EngineType.Pool: "POOL_",
    mybir.EngineType.SP: "SP_",
}


def _legalize_waits(nc: bass.Bass) -> None:
    """Walrus codegen allows a single sync-wait slot per ISA instruction;
    Tile can emit several.  Two-step legalization:

    1. Drop waits on the instruction's *own* engine completion semaphore when
       other waits are present (engines execute serially, so Tile's same-
       engine WAW guard is implied by program order).
    2. Hoist any remaining extra waits onto standalone EventSemaphore
       instructions inserted just before the owner on the same engine.
    """
    n_split = 0
    for f in nc.m.functions:
        for bb in f.blocks:
            new_insts = []
            for ins in bb.instructions:
                si = getattr(ins, "sync_info", None)
                if si is None or len(si.on_wait) < 2:
                    new_insts.append(ins)
                    continue
                waits = list(si.on_wait)
                prefix = _ENGINE_SEM_PREFIX.get(ins.engine)
                if prefix is not None:
                    kept = [w for w in waits if not (w.ant_name or "").startswith(prefix)]
                    if kept:
                        waits = kept
                for w in waits[:-1]:
                    ev = mybir.InstEventSemaphore(
                        name=f"W-split-{n_split}", ins=[], outs=[]
                    )
                    n_split += 1
                    ev.engine = ins.engine
                    # a +0 on the waited-on semaphore is semantically a no-op
                    # but satisfies the sim's "every instruction updates
                    # something" invariant
                    ev.sync_info = mybir.SyncInfo(
                        on_wait=[w],
                        on_update=[
                            mybir.SyncUpdate(
                                sync_type="semaphore",
                                id=w.id,
                                ant_name=w.ant_name,
                                update_mode="sem-add-imm",
                                update_value=0,
                            )
                        ],
                    )
                    new_insts.append(ev)
                ins.sync_info = mybir.SyncInfo(
                    on_wait=[waits[-1]], on_update=list(si.on_update)
                )
                new_insts.append(ins)
            bb.instructions[:] = new_insts


def build_nc(nt: int = NT, n_cores: int = N_CORES) -> bass.Bass:
    """Build the SPMD Bass program (one NEFF, run on all cores)."""
    nc = bass.Bass()

    # Per-core inputs/outputs. tgt[p, j] = target of local edge j*128 + p.
    tgt_in = nc.dram_tensor("tgt", [P, nt], i32, kind="ExternalInput")
    out_ext = nc.dram_tensor("out", [ROWS_PER_CORE, D_FEAT], f32, kind="ExternalOutput")

    with tile.TileContext(nc, num_cores=n_cores) as tc:
        with (
            tc.tile_pool(name="sbuf", bufs=1) as sb,
            tc.tile_pool(name="onehot", bufs=8) as oh,
            tc.tile_pool(name="outp", bufs=3) as op_pool,
            tc.tile_pool(name="psum", bufs=1, space="PSUM") as ps,
            tc.tile_pool(name="psum2", bufs=2, space="PSUM") as ps2,
            tc.tile_pool(name="dram", bufs=1, space="DRAM") as dram,
        ):
            # --- load targets, split into digits ---------------------------
            tgt_sb = sb.tile([P, nt], i32)
            nc.sync.dma_start(out=tgt_sb[:], in_=tgt_in[:])

            hi32 = sb.tile([P, nt], i32)
            lo32 = sb.tile([P, nt], i32)
            nc.vector.tensor_scalar(
                out=hi32[:], in0=tgt_sb[:], scalar1=7, scalar2=None,
                op0=mybir.AluOpType.logical_shift_right,
            )
            nc.vector.tensor_scalar(
                out=lo32[:], in0=tgt_sb[:], scalar1=127, scalar2=None,
                op0=mybir.AluOpType.bitwise_and,
            )
            # digit scalars for tensor_scalar (ISA wants fp32 scalar operands)
            hi_f = sb.tile([P, nt], f32)
            lo_f = sb.tile([P, nt], f32)
            nc.vector.tensor_copy(out=hi_f[:], in_=hi32[:])
            nc.vector.tensor_copy(out=lo_f[:], in_=lo32[:])

            iota_hi_i = sb.tile([P, HI], i32)
            iota_lo_i = sb.tile([P, LO], i32)
            nc.gpsimd.iota(iota_hi_i[:], pattern=[[1, HI]], base=0, channel_multiplier=0)
            nc.gpsimd.iota(iota_lo_i[:], pattern=[[1, LO]], base=0, channel_multiplier=0)
            iota_hi = sb.tile([P, HI], bf16)
            iota_lo = sb.tile([P, LO], bf16)
            nc.vector.tensor_copy(out=iota_hi[:], in_=iota_hi_i[:])
            nc.vector.tensor_copy(out=iota_lo[:], in_=iota_lo_i[:])

            # --- one-hots (tensor_scalar: contiguous 16-bit operands -> DVE
            # 4x packed mode; every POOL_EVERY-th tile built on GpSimd) + PE
            # matmul accumulation.  countsT[lo, hi] += B^T A per tile; the
            # contiguous 128-wide bf16 lhsT=B enables fast weight load, and
            # only 80 rhs columns stream per matmul.
            counts_t_ps = ps.tile([LO, HI], f32, space="PSUM")
            for j in range(nt):
                eng = nc.gpsimd if (j % POOL_EVERY == POOL_EVERY - 1) else nc.vector
                a_t = oh.tile([P, HI], bf16, tag="a")
                b_t = oh.tile([P, LO], bf16, tag="b")
                eng.tensor_scalar(
                    out=a_t[:], in0=iota_hi[:], scalar1=hi_f[:][:, j : j + 1],
                    scalar2=None, op0=mybir.AluOpType.is_equal,
                )
                eng.tensor_scalar(
                    out=b_t[:], in0=iota_lo[:], scalar1=lo_f[:][:, j : j + 1],
                    scalar2=None, op0=mybir.AluOpType.is_equal,
                )
                nc.tensor.matmul(
                    out=counts_t_ps[:],
                    lhsT=b_t[:],
                    rhs=a_t[:],
                    start=(j == 0),
                    stop=(j == nt - 1),
                )

            # transpose countsT -> counts[hi, lo] so the ReduceScatter chunks
            # are node-contiguous
            ident_g = sb.tile([P, P], f32)
            make_identity(nc, ident_g[:])
            ident = sb.tile([P, P], f32)
            nc.vector.tensor_copy(out=ident[:], in_=ident_g[:])
            counts_t_sb = sb.tile([LO, HI], f32)
            nc.vector.tensor_copy(out=counts_t_sb[:], in_=counts_t_ps[:])
            counts_ps2 = ps2.tile([HI, LO], f32, space="PSUM")
            nc.tensor.transpose(
                out=counts_ps2[:], in_=counts_t_sb[:], identity=ident[:]
            )
            counts_sb = sb.tile([HI, LO], f32)
            nc.vector.tensor_copy(out=counts_sb[:], in_=counts_ps2[:])

            # --- combine partial histograms across the 8 cores -------------
            cc_in = dram.tile([HI, LO], f32)
            cc_out = dram.tile([HI // n_cores, LO], f32)
            nc.sync.dma_start(out=cc_in[:], in_=counts_sb[:])
            nc.gpsimd.collective_compute(
                "ReduceScatter",
                mybir.AluOpType.add,
                replica_groups=[list(range(n_cores))],
                ins=[cc_in[:]],
                outs=[cc_out[:]],
            )
            # this core's slice: counts for nodes [core*1280, (core+1)*1280)
            nch = HI // n_cores
            chunk_raw = sb.tile([nch, LO], f32)
            nc.sync.dma_start(out=chunk_raw[:], in_=cc_out[:])

            # --- transpose so node-within-tile lands on partitions ---------
            # (operands routed through DVE so the transpose waits on a single
            # semaphore: the LdWeights ISA slot fits only one wait)
            chunk_sb = sb.tile([nch, LO], f32)
            nc.vector.tensor_copy(out=chunk_sb[:], in_=chunk_raw[:])
            deg_t_ps = ps2.tile([P, nch], f32, space="PSUM")
            nc.tensor.transpose(
                out=deg_t_ps[:], in_=chunk_sb[:], identity=ident[:][:nch, :nch]
            )
            deg_t = sb.tile([P, HI // n_cores], f32)
            nc.vector.tensor_copy(out=deg_t[:], in_=deg_t_ps[:])

            # --- emit output rows: 1.0 where deg > 0 -----------------------
            # one wide SBUF tile, one strided DMA (a single HW-DGE queue +
            # single wait; 10 separate DMAs would exceed the 8 queues and pick
            # up a second, unencodable queue-reuse wait)
            o_all = op_pool.tile([P, OUT_TILES * D_FEAT], f32)
            for k in range(OUT_TILES):
                nc.vector.tensor_scalar(
                    out=o_all[:][:, k * D_FEAT : (k + 1) * D_FEAT],
                    in0=deg_t[:][:, k : k + 1].to_broadcast([P, D_FEAT]),
                    scalar1=0.0,
                    scalar2=None,
                    op0=mybir.AluOpType.is_gt,
                )
            nc.sync.dma_start(
                out=out_ext[:].rearrange("(k p) f -> p k f", p=P),
                in_=o_all[:].rearrange("p (k f) -> p k f", f=D_FEAT),
            )

    _legalize_waits(nc)
    return nc


_NC_CACHE: dict = {}


def kernel(**inputs: np.ndarray) -> np.ndarray:
    global LAST_RESULTS
    edge_index = np.asarray(inputs["edge_index"])
    assert edge_index.shape == (2, N_EDGES), edge_index.shape
    tgt = np.ascontiguousarray(edge_index[1].astype(np.int32))

    key = (NT, N_CORES)
    if key not in _NC_CACHE:
        _NC_CACHE[key] = build_nc()
    nc = _NC_CACHE[key]

    in_maps = []
    for c in range(N_CORES):
        shard = np.full((E_PAD,), PAD_NODE, np.int32)
        shard[:E_LOC] = tgt[c * E_LOC : (c + 1) * E_LOC]
        shard = shard.reshape(NT, P).T
        in_maps.append({"tgt": np.ascontiguousarray(shard)})

    trace = bool(int(os.environ.get("KERNEL_TRACE", "0")))
    if trace:
        _ensure_ntff_hook()
    res = run_bass_kernel_spmd(
        nc,
        in_maps,
        core_ids=list(range(N_CORES)),
        trace=trace,
    )
    LAST_RESULTS = res

    out = np.concatenate([res.results[c]["out"] for c in range(N_CORES)], axis=0)
    return np.ascontiguousarray(out[:N_NODES]).astype(np.float32)


if __name__ == "__main__":
    # quick self-test with random inputs (no reference needed)
    rng = np.random.default_rng(0)
    ei = rng.integers(0, N_NODES, size=(2, N_EDGES)).astype(np.int32)
    x = rng.standard_normal((N_EDGES, D_FEAT)).astype(np.float32)
    out = kernel(source_node_representation_with_coefficient=x, edge_index=ei)
    deg = np.bincount(ei[1], minlength=N_NODES)
    exp = (deg > 0).astype(np.float32)[:, None] * np.ones((1, D_FEAT), np.float32)
    print("match:", np.array_equal(out, exp), "out mean:", out.mean())


# revision 26
# speedup vs baseline: 4.5750x; 4.5750x over previous
"""Trainium2 Bass kernel for nn_Aggregation_74904229642960 (gnn_message_passing).

The reference computes, with tgt = edge_index[1]:

    sm  = segment_softmax(x, tgt, N)   # per-(target node, feature) softmax over edges
    out = segment_sum(sm, tgt, N)      # [N, d]

The final segment_sum contracts exactly the segments the softmax normalized
over, and softmax weights sum to 1 over their own segment.  Hence, exactly
(independent of x, which only shifts/scales terms that cancel):

    out[n, f] = 1.0  if node n has >= 1 incoming edge, else 0.0

(The fp32 reference deviates from 1.0 by < 1e-6 rounding noise.)  The optimal
kernel therefore reads only edge_index[1]: it computes the in-degree histogram
(bincount over the 10000 nodes) on device and emits 1.0 rows for nodes with
nonzero degree.

Sharding (8 NeuronCores): edges are split E/8 per core (the E dim of
edge_index), each core builds a partial per-node histogram, the partials are
combined with a ReduceScatter(add) collective, and each core writes its 1/8
slice of the [N, d] output, which the host concatenates.

Per-core bincount (E_loc = 80000 edges), using node id n = hi*128 + lo:
  for each tile of 128 edges (one edge per SBUF partition):
      A[e, :] = onehot80(hi_e)    # DVE is_equal against iota, bf16
      B[e, :] = onehot128(lo_e)
      counts[hi, lo] += A^T @ B   # PE matmul, fp32 PSUM accumulation
  counts[hi, lo] == #edges with target hi*128+lo   (exact: 0/1 products,
  fp32 accumulate, counts <= 80000 << 2^24)
"""

import os

import numpy as np

import concourse.bass as bass
import concourse.mybir as mybir
import concourse.tile as tile
from concourse.bass_utils import run_bass_kernel_spmd
from concourse.masks import make_identity

N_NODES = 10000
N_EDGES = 640000
D_FEAT = 128
N_CORES = 8

P = 128               # SBUF partitions / edges per tile
HI = 80               # hi-digit one-hot width (hi = n >> 7 in [0, 79))
LO = 128              # lo-digit one-hot width (lo = n & 127)
NODES_PAD = HI * LO   # 10240 >= N_NODES
ROWS_PER_CORE = NODES_PAD // N_CORES      # 1280 output rows per core
OUT_TILES = ROWS_PER_CORE // P            # 10 output tiles of 128 nodes

E_LOC = N_EDGES // N_CORES                # 80000 real edges per core
NT = 640                                  # padded edge tiles per core (640*128 = 81920)
E_PAD = NT * P
PAD_NODE = 79 * LO                        # padding target (>= N_NODES, host-trimmed)
GRP = 32                                  # tiles per DVE one-hot group (even: keeps the
                                          # packed 2x DVE mode 4B-aligned)

f32 = mybir.dt.float32
bf16 = mybir.dt.bfloat16
i16 = mybir.dt.int16
i32 = mybir.dt.int32

# run_bass_kernel_spmd results of the most recent kernel() call (for test
# harness introspection: exec_time_ns etc. when BASS_TRACE=1).
LAST_RESULTS = None


def _ensure_ntff_hook():
    """Install the axon NTFF-profile hook if the container's antenv stub
    lacks it (profiling-only; kernel correctness does not depend on this)."""
    import sys
    import types

    try:
        from antenv.axon_hooks import get_axon_ntff_profile_hook  # noqa: F401

        return
    except ImportError:
        pass
    m = types.ModuleType("antenv.axon_hooks")
    m._hook = None
    m.set_axon_ntff_profile_hook = lambda h: setattr(m, "_hook", h)
    m.get_axon_ntff_profile_hook = lambda: m._hook
    import antenv

    sys.modules["antenv.axon_hooks"] = m
    antenv.axon_hooks = m
    try:
        from trn_agent_boot.trn_boot import _ntff_profile_via_ctypes

        hook = _ntff_profile_via_ctypes("/opt/axon/libaxon_pjrt.so")
        if hook is not None:
            m._hook = hook
    except Exception as e:  # profiling is best-effort
        print("ntff hook install failed:", e)


_ENGINE_SEM_PREFIX = {
    mybir.EngineType.PE: "PE_",
    mybir.EngineType.DVE: "DVE_",
    mybir.EngineType.Activation: "ACT_",
    mybir.EngineType.Pool: "POOL_",
    mybir.EngineType.SP: "SP_",
}


def _legalize_waits(nc: bass.Bass) -> None:
    """Walrus codegen allows a single sync-wait slot per ISA instruction;
    Tile can emit several.  Two-step legalization:

    1. Drop waits on the instruction's *own* engine completion semaphore when
       other waits are present (engines execute serially, so Tile's same-
       engine WAW guard is implied by program order).
    2. Hoist any remaining extra waits onto standalone EventSemaphore
       instructions inserted just before the owner on the same engine.
    """
    n_split = 0
    for f in nc.m.functions:
        for bb in f.blocks:
            new_insts = []
            for ins in bb.instructions:
                si = getattr(ins, "sync_info", None)
                if si is None or len(si.on_wait) < 2:
                    new_insts.append(ins)
                    continue
                waits = list(si.on_wait)
                prefix = _ENGINE_SEM_PREFIX.get(ins.engine)
                if prefix is not None:
                    kept = [w for w in waits if not (w.ant_name or "").startswith(prefix)]
                    if kept:
                        waits = kept
                for w in waits[:-1]:
                    ev = mybir.InstEventSemaphore(
                        name=f"W-split-{n_split}", ins=[], outs=[]
                    )
                    n_split += 1
                    ev.engine = ins.engine
                    # a +0 on the waited-on semaphore is semantically a no-op
                    # but satisfies the sim's "every instruction updates
                    # something" invariant
                    ev.sync_info = mybir.SyncInfo(
                        on_wait=[w],
                        on_update=[
                            mybir.SyncUpdate(
                                sync_type="semaphore",
                                id=w.id,
                                ant_name=w.ant_name,
                                update_mode="sem-add-imm",
                                update_value=0,
                            )
                        ],
                    )
                    new_insts.append(ev)
                ins.sync_info = mybir.SyncInfo(
                    on_wait=[waits[-1]], on_update=list(si.on_update)
                )
                new_insts.append(ins)
            bb.instructions[:] = new_insts


def build_nc(nt: int = NT, grp: int = GRP, n_cores: int = N_CORES) -> bass.Bass:
    """Build the SPMD Bass program (one NEFF, run on all cores)."""
    nc = bass.Bass()

    # Per-core inputs/outputs. tgt[p, j] = target of local edge j*128 + p.
    tgt_in = nc.dram_tensor("tgt", [P, nt], i32, kind="ExternalInput")
    out_ext = nc.dram_tensor("out", [ROWS_PER_CORE, D_FEAT], f32, kind="ExternalOutput")

    with tile.TileContext(nc, num_cores=n_cores) as tc:
        with (
            tc.tile_pool(name="sbuf", bufs=1) as sb,
            tc.tile_pool(name="onehot", bufs=8) as oh,
            tc.tile_pool(name="outp", bufs=3) as op_pool,
            tc.tile_pool(name="psum", bufs=1, space="PSUM") as ps,
            tc.tile_pool(name="psum2", bufs=2, space="PSUM") as ps2,
            tc.tile_pool(name="dram", bufs=1, space="DRAM") as dram,
        ):
            # --- load targets, split into digits ---------------------------
            tgt_sb = sb.tile([P, nt], i32)
            nc.sync.dma_start(out=tgt_sb[:], in_=tgt_in[:])

            hi32 = sb.tile([P, nt], i32)
            lo32 = sb.tile([P, nt], i32)
            nc.vector.tensor_scalar(
                out=hi32[:], in0=tgt_sb[:], scalar1=7, scalar2=None,
                op0=mybir.AluOpType.logical_shift_right,
            )
            nc.vector.tensor_scalar(
                out=lo32[:], in0=tgt_sb[:], scalar1=127, scalar2=None,
                op0=mybir.AluOpType.bitwise_and,
            )
            # digit scalars for tensor_scalar (ISA wants fp32 scalar operands)
            # digits in bf16 (exact: values <= 127)
            hi_sb = sb.tile([P, nt], bf16)
            lo_sb = sb.tile([P, nt], bf16)
            nc.vector.tensor_copy(out=hi_sb[:], in_=hi32[:])
            nc.vector.tensor_copy(out=lo_sb[:], in_=lo32[:])

            # iota for A is tiled [m, j] (m repeated grp times) so the A
            # one-hot op can keep j innermost (step 1 everywhere -> DVE 2x
            # packed mode); iota for B is plain [m] (B is built m-inner so the
            # matmul's stationary operand stays contiguous for FWL)
            iota_hi_i = sb.tile([P, HI * grp], i32)
            iota_lo_i = sb.tile([P, LO], i32)
            nc.gpsimd.iota(
                iota_hi_i[:], pattern=[[1, HI], [0, grp]], base=0, channel_multiplier=0
            )
            nc.gpsimd.iota(iota_lo_i[:], pattern=[[1, LO]], base=0, channel_multiplier=0)
            iota_hi = sb.tile([P, HI * grp], bf16)
            iota_lo = sb.tile([P, LO], bf16)
            nc.vector.tensor_copy(out=iota_hi[:], in_=iota_hi_i[:])
            nc.vector.tensor_copy(out=iota_lo[:], in_=iota_lo_i[:])

            # --- one-hots (DVE) + matmul accumulation (PE) -----------------
            # countsT[lo, hi] += B^T A per tile of 128 edges.  B (the
            # stationary matmul operand) is built m-inner: contiguous 128-wide
            # bf16 weights -> fast weight load; A (the moving operand) is
            # built j-inner for the DVE 2x packed mode, the strided rhs read
            # only costs the 2-per-cycle column packing.
            counts_t_ps = ps.tile([LO, HI], f32, space="PSUM")
            for g in range(nt // grp):
                j0 = g * grp
                a_grp = oh.tile([P, HI * grp], bf16, tag="a")
                b_grp = oh.tile([P, LO * grp], bf16, tag="b")
                # A[p, (m, j)] = (hi[p, j0+j] == m); j innermost
                nc.vector.tensor_tensor(
                    out=a_grp[:].rearrange("p (m j) -> p m j", j=grp),
                    in0=hi_sb[:][:, j0 : j0 + grp][:, None, :].to_broadcast(
                        [P, HI, grp]
                    ),
                    in1=iota_hi[:].rearrange("p (m j) -> p m j", j=grp),
                    op=mybir.AluOpType.is_equal,
                )
                # B[p, (j, m)] = (lo[p, j0+j] == m); m innermost
                nc.vector.tensor_tensor(
                    out=b_grp[:].rearrange("p (j m) -> p j m", m=LO),
                    in0=lo_sb[:][:, j0 : j0 + grp][:, :, None].to_broadcast(
                        [P, grp, LO]
                    ),
                    in1=iota_lo[:][:, None, :].to_broadcast([P, grp, LO]),
                    op=mybir.AluOpType.is_equal,
                )
                a_t = a_grp[:].rearrange("p (m j) -> p m j", j=grp)
                for j in range(grp):
                    jj = j0 + j
                    nc.tensor.matmul(
                        out=counts_t_ps[:],
                        lhsT=b_grp[:][:, j * LO : (j + 1) * LO],
                        rhs=a_t[:, :, j],
                        start=(jj == 0),
                        stop=(jj == nt - 1),
                    )

            # transpose countsT -> counts[hi, lo] so the ReduceScatter chunks
            # are node-contiguous
            ident_g = sb.tile([P, P], f32)
            make_identity(nc, ident_g[:])
            ident = sb.tile([P, P], f32)
            nc.vector.tensor_copy(out=ident[:], in_=ident_g[:])
            counts_t_sb = sb.tile([LO, HI], f32)
            nc.vector.tensor_copy(out=counts_t_sb[:], in_=counts_t_ps[:])
            counts_ps2 = ps2.tile([HI, LO], f32, space="PSUM")
            nc.tensor.transpose(
                out=counts_ps2[:], in_=counts_t_sb[:], identity=ident[:]
            )
            counts_sb = sb.tile([HI, LO], f32)
            nc.vector.tensor_copy(out=counts_sb[:], in_=counts_ps2[:])

            # --- combine partial histograms across the 8 cores -------------
            cc_in = dram.tile([HI, LO], f32)
            cc_out = dram.tile([HI // n_cores, LO], f32)
            nc.sync.dma_start(out=cc_in[:], in_=counts_sb[:])
            nc.gpsimd.collective_compute(
                "ReduceScatter",
                mybir.AluOpType.add,
                replica_groups=[list(range(n_cores))],
                ins=[cc_in[:]],
                outs=[cc_out[:]],
            )
            # this core's slice: counts for nodes [core*1280, (core+1)*1280)
            nch = HI // n_cores
            chunk_raw = sb.tile([nch, LO], f32)
            nc.sync.dma_start(out=chunk_raw[:], in_=cc_out[:])

            # --- transpose so node-within-tile lands on partitions ---------
            # (operands routed through DVE so the transpose waits on a single
            # semaphore: the LdWeights ISA slot fits only one wait)
            chunk_sb = sb.tile([nch, LO], f32)
            nc.vector.tensor_copy(out=chunk_sb[:], in_=chunk_raw[:])
            deg_t_ps = ps2.tile([P, nch], f32, space="PSUM")
            nc.tensor.transpose(
                out=deg_t_ps[:], in_=chunk_sb[:], identity=ident[:][:nch, :nch]
            )
            deg_t = sb.tile([P, HI // n_cores], f32)
            nc.vector.tensor_copy(out=deg_t[:], in_=deg_t_ps[:])

            # --- emit output rows: 1.0 where deg > 0 -----------------------
            # one wide SBUF tile, one strided DMA (a single HW-DGE queue +
            # single wait; 10 separate DMAs would exceed the 8 queues and pick
            # up a second, unencodable queue-reuse wait)
            o_all = op_pool.tile([P, OUT_TILES * D_FEAT], f32)
            for k in range(OUT_TILES):
                nc.vector.tensor_scalar(
                    out=o_all[:][:, k * D_FEAT : (k + 1) * D_FEAT],
                    in0=deg_t[:][:, k : k + 1].to_broadcast([P, D_FEAT]),
                    scalar1=0.0,
                    scalar2=None,
                    op0=mybir.AluOpType.is_gt,
                )
            nc.sync.dma_start(
                out=out_ext[:].rearrange("(k p) f -> p k f", p=P),
                in_=o_all[:].rearrange("p (k f) -> p k f", f=D_FEAT),
            )

    _legalize_waits(nc)
    return nc


_NC_CACHE: dict = {}


def kernel(**inputs: np.ndarray) -> np.ndarray:
    global LAST_RESULTS
    edge_index = np.asarray(inputs["edge_index"])
    assert edge_index.shape == (2, N_EDGES), edge_index.shape
    tgt = np.ascontiguousarray(edge_index[1].astype(np.int32))

    key = (NT, GRP, N_CORES)
    if key not in _NC_CACHE:
        _NC_CACHE[key] = build_nc()
    nc = _NC_CACHE[key]

    in_maps = []
    for c in range(N_CORES):
        shard = np.full((E_PAD,), PAD_NODE, np.int32)
        shard[:E_LOC] = tgt[c * E_LOC : (c + 1) * E_LOC]
        shard = shard.reshape(NT, P).T
        in_maps.append({"tgt": np.ascontiguousarray(shard)})

    trace = bool(int(os.environ.get("KERNEL_TRACE", "0")))
    if trace:
        _ensure_ntff_hook()
    res = run_bass_kernel_spmd(
        nc,
        in_maps,
        core_ids=list(range(N_CORES)),
        trace=trace,
    )
    LAST_RESULTS = res

    out = np.concatenate([res.results[c]["out"] for c in range(N_CORES)], axis=0)
    return np.ascontiguousarray(out[:N_NODES]).astype(np.float32)


if __name__ == "__main__":
    # quick self-test with random inputs (no reference needed)
    rng = np.random.default_rng(0)
    ei = rng.integers(0, N_NODES, size=(2, N_EDGES)).astype(np.int32)
    x = rng.standard_normal((N_EDGES, D_FEAT)).astype(np.float32)
    out = kernel(source_node_representation_with_coefficient=x, edge_index=ei)
    deg = np.bincount(ei[1], minlength=N_NODES)
    exp = (deg > 0).astype(np.float32)[:, None] * np.ones((1, D_FEAT), np.float32)
    print("match:", np.array_equal(out, exp), "out mean:", out.mean())
